# revision 33
# baseline (speedup 1.0000x reference)
"""Trainium2 Bass kernel for a 3-layer mean-aggregation GNN (BehavioralGNN).

Math per conv layer (reference):
    agg = segment_sum(x[src], dst) / max(deg, 1)
    x'  = relu((agg + x) @ W.T + b)
Heads: next_event = x3 @ Wp.T + bp ; event_classes = x3 @ Wc.T + bc

Distribution: nodes are partitioned across 8 NeuronCores (each core owns a
node shard and its incoming edges).  Remote source features are obtained by
AllGather-ing the per-shard tables between layers.

Device-side plan (per core):
  * Nodes are packed into "windows" of <=128 nodes whose incoming edge count
    (incl. one self-edge per node) fits K*128 slots.  All tensors live in a
    window-major permuted layout [NW*128, F] so every write is a direct DMA;
    the host un-permutes at the end.
  * Layers 1,2 are computed "W-first": y = x @ W.T is computed densely per
    window (cheap), AllGather(y), and the edge aggregation then runs in the
    output feature space:  x' = relu(seg_sum(w_e * y[src]) + b)  where
    w_e = 1/deg for real edges and w_e = 1 for the self-edge (this folds the
    "+x" term and the degree division into the segment-sum).
  * The segment-sum is a TensorEngine matmul per 128-edge chunk:
    PSUM[F, 128nodes] += g_chunk.T @ S_chunk, where g is the indirect-DMA
    gather of y[src] rows and S[p, j] = (j == dst_local[p]) * w_e[p] is
    built on DVE with a single fused tensor_scalar(is_equal, mult) using a
    constant iota tile.
  * Layer 3 aggregates x3 itself (x-first) and applies W3 + relu + the two
    heads per window; head outputs are written transposed and fixed up on
    the host.
"""

import os
import sys

import numpy as np

for _p in (
    "/opt/trn_rl_repo",
    "/opt/pypackages",
    "/root/.axon_site/_ro/trn_rl_repo",
    "/root/.axon_site/_ro/pypackages",
):
    if os.path.isdir(_p) and _p not in sys.path:
        sys.path.append(_p)

import concourse.bass as bass
import concourse.bacc as bacc
import concourse.mybir as mybir
import concourse.tile as tile
from concourse.bass import IndirectOffsetOnAxis

# ---------------------------------------------------------------- problem
N_NODES = 50000
N_EDGES = 800000
N_CORES = 8
DIMS = [64, 128, 128, 64]  # input -> h1 -> h2 -> h3
N_CLASSES = 10
P = 128

# ---------------------------------------------------------------- config
K_CANDIDATES = (16, 17, 18, 19, 20)  # chunks per window, auto-picked
TABLE_DT = mybir.dt.float32          # dtype of gather tables / gathered rows
MM_VIEW_DT = None                    # e.g. mybir.dt.float32r to bitcast matmuls
NP_TABLE_DT = np.float32


def _f32(x):
    return np.ascontiguousarray(x, dtype=np.float32)


# =================================================================== prep
def _pack(deg, budget, shard):
    """Greedy node->window packing.  Returns list of (a, b) local node ranges."""
    costs = deg + 1  # + self edge
    cum = np.concatenate([[0], np.cumsum(costs)])
    wins = []
    a = 0
    while a < shard:
        b = min(a + P, shard)
        b2 = int(np.searchsorted(cum, cum[a] + budget, side="right")) - 1
        b = max(a + 1, min(b, b2))
        wins.append((a, b))
        a = b
    return wins


def prepare(node_features, edge_index, n_nodes=N_NODES, n_cores=N_CORES):
    """Host-side graph preprocessing.  Only touches edge structure (+ builds
    the windowed transpose of the input features).

    The device gathers run through dma_gather whose indices are int16, so the
    [n_cores*NW*128]-row tables are addressed as two halves (lo/hi); each
    window's edges are split into a lo-stream and a hi-stream of 128-edge
    chunks.  K_lo[w]/K_hi[w] (max over cores) define the shared program
    structure; per-core shortfalls are padded with weight-0 edges that gather
    row 0."""
    shard = n_nodes // n_cores
    src = edge_index[0].astype(np.int64)
    dst = edge_index[1].astype(np.int64)
    deg = np.bincount(dst, minlength=n_nodes)
    invdeg = (1.0 / np.maximum(deg, 1)).astype(np.float64)

    order = np.argsort(dst, kind="stable")
    ds = dst[order]
    ss = src[order]
    estart = np.searchsorted(ds, np.arange(n_nodes + 1))  # per-node edge range

    # pass 1: windows per core for each K candidate; pick K minimizing NW*K
    best = None
    for K in K_CANDIDATES:
        wins_all = [
            _pack(deg[c * shard:(c + 1) * shard], K * P, shard)
            for c in range(n_cores)
        ]
        nw = max(len(w) for w in wins_all)
        if best is None or nw * K < best[0] * best[1]:
            best = (nw, K, wins_all)
    NW, K, wins_all = best
    HALF = n_cores * NW * P // 2

    # pass 2: global permuted id for every node: perm[n] = c*NW*P + w*P + j
    perm = np.zeros(n_nodes, dtype=np.int64)
    for c in range(n_cores):
        for w, (a, b) in enumerate(wins_all[c]):
            perm[c * shard + a: c * shard + b] = (
                c * NW * P + w * P + np.arange(b - a)
            )

    # pass 3: per-(core,window) lo/hi edge lists
    _e64 = np.zeros(0, dtype=np.int64)
    _ef = np.zeros(0, dtype=np.float32)
    _empty = ((_e64, _ef, _ef), (_e64, _ef, _ef))
    raw = [[_empty] * NW for _ in range(n_cores)]
    n_lo = np.zeros((n_cores, NW), dtype=np.int64)
    n_hi = np.zeros((n_cores, NW), dtype=np.int64)
    xtps = []
    for c in range(n_cores):
        lo_e, hi_e = estart[c * shard], estart[(c + 1) * shard]
        nodes_g = np.arange(c * shard, (c + 1) * shard)
        ins_pos = estart[c * shard:(c + 1) * shard] - lo_e
        # interleave one self-edge (weight 1) before each node's real edges
        sfull = perm[np.insert(ss[lo_e:hi_e], ins_pos, nodes_g)]
        dfull = np.insert(ds[lo_e:hi_e] - c * shard, ins_pos, np.arange(shard))
        wfull = np.insert(invdeg[ds[lo_e:hi_e]], ins_pos, 1.0).astype(np.float32)
        ebound = np.concatenate(
            [ins_pos + np.arange(shard), [hi_e - lo_e + shard]]
        )
        xtp = np.zeros((DIMS[0], NW * P), dtype=np.float32)
        for w, (a, b) in enumerate(wins_all[c]):
            ea, eb = ebound[a], ebound[b]
            s_w = sfull[ea:eb]
            d_w = (dfull[ea:eb] - a).astype(np.float32)
            v_w = wfull[ea:eb]
            m = s_w < HALF
            raw[c][w] = (
                (s_w[m], d_w[m], v_w[m]),
                (s_w[~m] - HALF, d_w[~m], v_w[~m]),
            )
            n_lo[c, w] = int(m.sum())
            n_hi[c, w] = int((~m).sum())
            xtp[:, w * P: w * P + (b - a)] = node_features[
                c * shard + a: c * shard + b
            ].T
        xtps.append(xtp)

    K_lo = [int(np.ceil(n_lo[:, w].max() / P)) for w in range(NW)]
    K_hi = [int(np.ceil(n_hi[:, w].max() / P)) for w in range(NW)]

    # pass 4: padded per-core tables following the shared chunk structure
    C_total = sum(K_lo) + sum(K_hi)
    cores = []
    for c in range(n_cores):
        dstloc = np.full((P, C_total), -1.0, dtype=np.float32)
        ew = np.zeros((P, C_total), dtype=np.float32)
        gi_lo = np.zeros((16, sum(K_lo) * 8), dtype=np.int16)
        gi_hi = np.zeros((16, sum(K_hi) * 8), dtype=np.int16)
        co = 0
        plo = phi = 0
        for w in range(NW):
            for (idx, dl, wv), Kw, gi, po in (
                (raw[c][w][0], K_lo[w], gi_lo, plo),
                (raw[c][w][1], K_hi[w], gi_hi, phi),
            ):
                L = len(idx)
                slots = Kw * P
                assert L <= slots
                if Kw:
                    ii = np.zeros(slots, dtype=np.int16)
                    dd = np.full(slots, -1.0, dtype=np.float32)
                    vv = np.zeros(slots, dtype=np.float32)
                    ii[:L] = idx
                    dd[:L] = dl
                    vv[:L] = wv
                    dstloc[:, co:co + Kw] = dd.reshape(Kw, P).T
                    ew[:, co:co + Kw] = vv.reshape(Kw, P).T
                    gi[:, po * 8:(po + Kw) * 8] = ii.reshape(-1, 16).T
                co += Kw
            plo += K_lo[w]
            phi += K_hi[w]
        gidx = np.tile(np.concatenate([gi_lo, gi_hi], axis=1), (8, 1))
        cores.append(dict(dstloc=dstloc, ew=ew, gidx=gidx, xtp=xtps[c]))

    return dict(
        NW=NW, K=K, K_lo=K_lo, K_hi=K_hi, C_total=C_total,
        perm=perm, cores=cores, shard=shard,
    )


# ---------------------------------------------------------------- consts
def const_layout(NW, C_total):
    """Column layout of the packed f32 constant tensor [128, CW].

    Packing everything into one tensor keeps the DMA-wait fan-in of the
    first consumer instructions at 1 (walrus rejects instructions with too
    many sync waits)."""
    F0, F1, F2, F3 = DIMS
    NWK = C_total
    off = {}
    c = 0

    def put(name, cols):
        nonlocal c
        off[name] = c
        c += cols

    put("xtp", NW * P)  # rows 0:F0
    put("dstloc", NWK)
    put("ew", NWK)
    put("iota", P)
    put("ident", P)
    put("w1t", F1)   # rows 0:F0
    put("w2t", F2)   # rows 0:F1
    put("w3t", F3)   # rows 0:F2
    put("wpt", F3)   # rows 0:F3
    put("wct", N_CLASSES)  # rows 0:F3
    put("b1c", 1)
    put("b2c", 1)
    put("b3c", 1)
    put("bpc", 1)
    put("bcc", 1)
    off["_total"] = c
    return off


# ================================================================ builder
GATHER_GROUP = 2  # windows per dma_gather call


def build_nc(prep, n_cores=N_CORES):
    f32 = mybir.dt.float32
    i16 = mybir.dt.int16
    F0, F1, F2, F3 = DIMS
    NW, K_lo, K_hi = prep["NW"], prep["K_lo"], prep["K_hi"]
    C_total = prep["C_total"]
    NWP = NW * P
    TBL = NWP * n_cores  # rows of an all-gathered table
    HALF = TBL // 2
    SK_lo = sum(K_lo)

    nc = bacc.Bacc("TRN2", num_devices=n_cores)

    # ---- parameters
    lay = const_layout(NW, C_total)
    gof = nc.declare_dram_parameter(
        "gidx", [P, (SK_lo + sum(K_hi)) * 8], i16, isOutput=False
    )
    cpk = nc.declare_dram_parameter("cpack", [P, lay["_total"]], f32, isOutput=False)

    emb_o = nc.declare_dram_parameter("embT", [F3, NWP], f32, isOutput=True)
    ne_o = nc.declare_dram_parameter("neT", [F3, NWP], f32, isOutput=True)
    ec_o = nc.declare_dram_parameter("ecT", [N_CLASSES, NWP], f32, isOutput=True)

    # ---- internal DRAM: per-layer shard tables + allgathered tables
    tdt = TABLE_DT
    y1s = nc.dram_tensor("y1s", [NWP, F1], tdt)
    y1f = nc.dram_tensor("y1f", [TBL, F1], tdt)
    y2s = nc.dram_tensor("y2s", [NWP, F2], tdt)
    y2f = nc.dram_tensor("y2f", [TBL, F2], tdt)
    # x3 = layer2 output has dim F2; layer3 aggregates x3 then applies W3.
    x3s = nc.dram_tensor("x3s", [NWP, F2], tdt)
    x3f = nc.dram_tensor("x3f", [TBL, F2], tdt)

    rg = [list(range(n_cores))]
    AF = mybir.ActivationFunctionType

    def mm_view(ap):
        return ap.bitcast(MM_VIEW_DT) if MM_VIEW_DT is not None else ap

    with tile.TileContext(nc) as tc:
        with (
            tc.tile_pool(name="const", bufs=1) as cp,
            tc.tile_pool(name="gath", bufs=2) as gp,
            tc.tile_pool(name="sel", bufs=4) as sp,
            tc.tile_pool(name="tmp", bufs=3) as xp_pool,
            tc.tile_pool(name="ps_h", bufs=2, space="PSUM") as ph,
            tc.tile_pool(name="ps_b", bufs=2, space="PSUM") as pb,
            tc.tile_pool(name="ps_c", bufs=2, space="PSUM") as pc,
        ):
            # ---------------- load constants into SBUF (2 DMAs total)
            gof_s = cp.tile([P, (SK_lo + sum(K_hi)) * 8], i16, tag="gidx")
            nc.sync.dma_start(out=gof_s[:], in_=gof[:])
            cpk_s = cp.tile([P, lay["_total"]], f32, tag="cpack")
            nc.sync.dma_start(out=cpk_s[:], in_=cpk[:])

            def cs(name, rows, cols):
                o = lay[name]
                return cpk_s[:rows, o:o + cols]

            xp_s = cs("xtp", F0, NWP)
            dsl_s = cs("dstloc", P, C_total)
            ew_s = cs("ew", P, C_total)
            iot_s = cs("iota", P, P)
            idn_s = cs("ident", P, P)
            w1t_s = cs("w1t", F0, F1)
            w2t_s = cs("w2t", F1, F2)
            w3t_s = cs("w3t", F2, F3)
            wpt_s = cs("wpt", F3, F3)
            wct_s = cs("wct", F3, N_CLASSES)
            b1_s = cs("b1c", F1, 1)
            b2_s = cs("b2c", F2, 1)
            b3_s = cs("b3c", F3, 1)
            bp_s = cs("bpc", F3, 1)
            bc_s = cs("bcc", N_CLASSES, 1)

            # ---------------- helpers
            def w_pass(w, rhs_ap, wt_s, kin, kout, dst_dram):
                """yT = wt.T @ xT per window; transpose; write natural rows to
                dst_dram[w*P:(w+1)*P, :]."""
                ps_yt = pb.tile([kout, P], f32, tag="ps_ab")
                nc.tensor.matmul(
                    out=ps_yt[:], lhsT=wt_s[:], rhs=rhs_ap, start=True, stop=True
                )
                yt_s = xp_pool.tile([kout, P], f32, tag="yt_s")
                nc.scalar.activation(yt_s[:], ps_yt[:], AF.Copy)
                ps_y = pb.tile([P, kout], f32, tag="ps_ab")
                nc.tensor.transpose(
                    out=ps_y[:], in_=yt_s[:], identity=idn_s[:kout, :kout]
                )
                y_s = xp_pool.tile([P, kout], tdt, tag="y_s")
                nc.vector.tensor_copy(out=y_s[:], in_=ps_y[:])
                nc.sync.dma_start(
                    out=dst_dram[w * P:(w + 1) * P, :], in_=y_s[:]
                )

            # chunk-column / gather-stream prefix offsets per window
            CO = np.concatenate(
                [[0], np.cumsum([K_lo[w] + K_hi[w] for w in range(NW)])]
            )
            PLO = np.concatenate([[0], np.cumsum(K_lo)])
            PHI = np.concatenate([[0], np.cumsum(K_hi)])

            def agg_layer(table, fdim, epilogue):
                """Per group of windows: two dma_gathers (lo/hi half-table),
                then per window the chunked segment-sum matmul into PSUM
                [fdim, P] followed by epilogue(w, ps_h)."""
                for w0 in range(0, NW, GATHER_GROUP):
                    grp = list(range(w0, min(w0 + GATHER_GROUP, NW)))
                    gklo = int(PLO[grp[-1] + 1] - PLO[w0])
                    gkhi = int(PHI[grp[-1] + 1] - PHI[w0])
                    g_lo = g_hi = None
                    if gklo:
                        g_lo = gp.tile([P, gklo * fdim], tdt, tag="glo")
                        nc.gpsimd.dma_gather(
                            g_lo[:].rearrange("p (c f) -> p c f", f=fdim),
                            table[0:HALF, :],
                            gof_s[:, int(PLO[w0]) * 8:int(PLO[w0] + gklo) * 8],
                            gklo * P,
                            gklo * P,
                            fdim,
                            single_packet=False,
                        )
                    if gkhi:
                        g_hi = gp.tile([P, gkhi * fdim], tdt, tag="ghi")
                        nc.gpsimd.dma_gather(
                            g_hi[:].rearrange("p (c f) -> p c f", f=fdim),
                            table[HALF:TBL, :],
                            gof_s[
                                :,
                                (SK_lo + int(PHI[w0])) * 8:
                                (SK_lo + int(PHI[w0]) + gkhi) * 8,
                            ],
                            gkhi * P,
                            gkhi * P,
                            fdim,
                            single_packet=False,
                        )
                    for w in grp:
                        nch = K_lo[w] + K_hi[w]
                        ps_h = ph.tile([fdim, P], f32, tag="ps_h")
                        ki = 0
                        for half, gt, pref in (
                            (K_lo[w], g_lo, int(PLO[w] - PLO[w0])),
                            (K_hi[w], g_hi, int(PHI[w] - PHI[w0])),
                        ):
                            for k in range(half):
                                ci = pref + k
                                col = int(CO[w]) + ki
                                s_t = sp.tile([P, P], tdt, tag="s")
                                nc.vector.tensor_scalar(
                                    out=s_t[:],
                                    in0=iot_s[:],
                                    scalar1=dsl_s[:, col:col + 1],
                                    scalar2=ew_s[:, col:col + 1],
                                    op0=mybir.AluOpType.is_equal,
                                    op1=mybir.AluOpType.mult,
                                )
                                nc.tensor.matmul(
                                    out=ps_h[:],
                                    lhsT=mm_view(
                                        gt[:, ci * fdim:(ci + 1) * fdim]
                                    ),
                                    rhs=mm_view(s_t[:]),
                                    start=(ki == 0),
                                    stop=(ki == nch - 1),
                                )
                                ki += 1
                        epilogue(w, ps_h)

            # ---------------- stage A: y1 = x1 @ W1.T  (window dense pass)
            for w in range(NW):
                w_pass(w, xp_s[:, w * P:(w + 1) * P], w1t_s, F0, F1, y1s)
            nc.gpsimd.collective_compute(
                "AllGather",
                mybir.AluOpType.bypass,
                replica_groups=rg,
                ins=[y1s[:, :]],
                outs=[y1f[:, :]],
            )

            # ---------------- stage C: layer1 agg + relu; fused y2 pass
            def epi_c(w, ps_h):
                x2t_s = xp_pool.tile([F1, P], f32, tag="x2t")
                nc.scalar.activation(x2t_s[:], ps_h[:], AF.Relu, bias=b1_s[:, :])
                w_pass(w, x2t_s[:], w2t_s, F1, F2, y2s)

            agg_layer(y1f, F1, epi_c)
            nc.gpsimd.collective_compute(
                "AllGather",
                mybir.AluOpType.bypass,
                replica_groups=rg,
                ins=[y2s[:, :]],
                outs=[y2f[:, :]],
            )

            # ---------------- stage E: layer2 agg + relu -> x3; write x3
            def epi_e(w, ps_h):
                x3t_s = xp_pool.tile([F2, P], f32, tag="x3t")
                nc.scalar.activation(x3t_s[:], ps_h[:], AF.Relu, bias=b2_s[:, :])
                # transpose to natural rows and store shard table
                ps_x3 = pb.tile([P, F2], f32, tag="ps_ab")
                nc.tensor.transpose(
                    out=ps_x3[:], in_=x3t_s[:], identity=idn_s[:, :]
                )
                x3_s = xp_pool.tile([P, F2], tdt, tag="y_s")
                nc.vector.tensor_copy(out=x3_s[:], in_=ps_x3[:])
                nc.sync.dma_start(out=x3s[w * P:(w + 1) * P, :], in_=x3_s[:])

            agg_layer(y2f, F2, epi_e)
            nc.gpsimd.collective_compute(
                "AllGather",
                mybir.AluOpType.bypass,
                replica_groups=rg,
                ins=[x3s[:, :]],
                outs=[x3f[:, :]],
            )

            # ---------------- stage G: layer3 agg (x-first) + W3 + heads
            def epi_g(w, ps_h):
                # ps_h [F2, P] = (agg/deg + x3).T
                h3t_s = xp_pool.tile([F2, P], f32, tag="h3t")
                nc.scalar.activation(h3t_s[:], ps_h[:], AF.Copy)
                # x4T = relu(W3 @ h3T + b3)
                ps_x4 = pc.tile([F3, P], f32, tag="ps_head")
                nc.tensor.matmul(
                    out=ps_x4[:], lhsT=w3t_s[:], rhs=h3t_s[:], start=True, stop=True
                )
                x4t_s = xp_pool.tile([F3, P], f32, tag="x4t")
                nc.scalar.activation(x4t_s[:], ps_x4[:], AF.Relu, bias=b3_s[:, :])
                nc.sync.dma_start(
                    out=emb_o[:, w * P:(w + 1) * P], in_=x4t_s[:]
                )
                # next_event head
                ps_ne = pc.tile([F3, P], f32, tag="ps_head")
                nc.tensor.matmul(
                    out=ps_ne[:], lhsT=wpt_s[:], rhs=x4t_s[:], start=True, stop=True
                )
                ne_s = xp_pool.tile([F3, P], f32, tag="ne_s")
                nc.vector.tensor_scalar(
                    out=ne_s[:],
                    in0=ps_ne[:],
                    scalar1=bp_s[:, :],
                    scalar2=None,
                    op0=mybir.AluOpType.add,
                )
                nc.sync.dma_start(out=ne_o[:, w * P:(w + 1) * P], in_=ne_s[:])
                # class head
                ps_ec = pc.tile([N_CLASSES, P], f32, tag="ps_head")
                nc.tensor.matmul(
                    out=ps_ec[:], lhsT=wct_s[:], rhs=x4t_s[:], start=True, stop=True
                )
                ec_s = xp_pool.tile([N_CLASSES, P], f32, tag="ec_s")
                nc.vector.tensor_scalar(
                    out=ec_s[:],
                    in0=ps_ec[:],
                    scalar1=bc_s[:, :],
                    scalar2=None,
                    op0=mybir.AluOpType.add,
                )
                nc.sync.dma_start(out=ec_o[:, w * P:(w + 1) * P], in_=ec_s[:])

            agg_layer(x3f, F2, epi_g)

    nc.finalize()
    return nc


# ================================================================ in_maps
def make_in_maps(prep, W1, b1, W2, b2, W3, b3, Wp, bp, Wc, bc):
    NW = prep["NW"]
    lay = const_layout(NW, prep["C_total"])
    NWK = prep["C_total"]

    base = np.zeros((P, lay["_total"]), dtype=np.float32)

    def put(name, arr):
        arr = _f32(arr)
        r, c = arr.shape
        base[:r, lay[name]:lay[name] + c] = arr

    put("iota", np.tile(np.arange(P, dtype=np.float32), (P, 1)))
    put("ident", np.eye(P, dtype=np.float32))
    put("w1t", np.asarray(W1).T)
    put("w2t", np.asarray(W2).T)
    put("w3t", np.asarray(W3).T)
    put("wpt", np.asarray(Wp).T)
    put("wct", np.asarray(Wc).T)
    put("b1c", np.asarray(b1).reshape(-1, 1))
    put("b2c", np.asarray(b2).reshape(-1, 1))
    put("b3c", np.asarray(b3).reshape(-1, 1))
    put("bpc", np.asarray(bp).reshape(-1, 1))
    put("bcc", np.asarray(bc).reshape(-1, 1))

    in_maps = []
    for cd in prep["cores"]:
        cpack = base.copy()
        cpack[:DIMS[0], lay["xtp"]:lay["xtp"] + NW * P] = cd["xtp"]
        cpack[:, lay["dstloc"]:lay["dstloc"] + NWK] = cd["dstloc"]
        cpack[:, lay["ew"]:lay["ew"] + NWK] = cd["ew"]
        in_maps.append(
            dict(
                gidx=np.ascontiguousarray(cd["gidx"], dtype=np.int16),
                cpack=cpack,
            )
        )
    return in_maps


def assemble(prep, out_maps, n_nodes=N_NODES):
    """Un-permute per-core windowed outputs back to node order."""
    perm = prep["perm"]
    embT = np.concatenate([m["embT"] for m in out_maps], axis=1)
    neT = np.concatenate([m["neT"] for m in out_maps], axis=1)
    ecT = np.concatenate([m["ecT"] for m in out_maps], axis=1)
    emb = np.ascontiguousarray(embT[:, perm].T)
    ne = np.ascontiguousarray(neT[:, perm].T)
    ec = np.ascontiguousarray(ecT[:, perm].T)
    return emb, ne, ec


# ================================================================= kernel
def kernel(
    node_features,
    edge_index,
    W1, b1, W2, b2, W3, b3, Wp, bp, Wc, bc,
):
    from concourse import bass_utils

    node_features = np.asarray(node_features, dtype=np.float32)
    edge_index = np.asarray(edge_index)

    prep = prepare(node_features, edge_index)
    nc = build_nc(prep)
    in_maps = make_in_maps(prep, W1, b1, W2, b2, W3, b3, Wp, bp, Wc, bc)
    res = bass_utils.run_bass_kernel_spmd(
        nc, in_maps, core_ids=list(range(N_CORES))
    )
    emb, ne, ec = assemble(prep, res.results)
    return (
        emb.astype(np.float32),
        ne.astype(np.float32),
        ec.astype(np.float32),
    )
